# revision 1
# baseline (speedup 1.0000x reference)
"""Multi-head attention (B=2, S=2048, H=1024, NH=16 heads of 64) on 8 trn2
NeuronCores, tensor-parallel over heads with batch parallelism on top.

Sharding: core c handles batch b=c//4 and head-group g=c%4 (4 heads, 256 of
the 1024 hidden cols). Each core computes the partial output
ctx_g @ Wo[g_rows, :]; the host sums the 4 partials per batch and adds the
closed-form bias terms (bv @ Wo + bo; bq/bk are applied on-device).

Device math (per core), in transposed-score space (fp16 matmul operands,
fp32 PSUM accumulation — fp16 is full-rate on the PE and ~5x more accurate
than bf16 here since all values are well within fp16 range):

  qT/kT = Wq_g^T x_b^T (+bias/partition)  [2 head-pair tiles of 128 x 2048]
  v     = x_b Wv_g                        [16 tiles of 128 x (4*65)]; col 64
                                          of each head block = 1.0, so the
                                          ctx matmul also accumulates the
                                          softmax denominators
  scoresT[k,q] = kT.T qT                  (PE; head pairs packed as
                                           [h0 512q | h1 512q] per 2-bank
                                           PSUM tile)
  expT = exp(0.125*scoresT + mask[k])     (one ACT op: scale+mask+exp; mask
                                           is a per-partition bias; no max
                                           subtraction needed -- |scores|<~4)
  ctxT_aug[c,q] += v_aug.T expT           (PE, accumulated over k in PSUM)
  normalize: PE-transpose ctxT to [q,c] (4 q-tiles packed per PSUM bank),
             strided reciprocal of the denominator column, per-partition mul
  out[q,:] = ctx_n @ Wo_g                 (PE, via transpose back to [c,q])

Schedule: single-kc software pipelining throughout (the PE is in-order, so
ctx matmuls trail their exp by 2 slots); phase 1 interleaves the projections
with attention on one head-pair; phase 2 runs one (pair, q-chunk) combo at a
time, with normalizes, output projections and the deferred q-projections
spliced into each combo's kc loop as PE filler while ACT (the exp engine) is
the steady-state bottleneck. Cost-model timeline: ~207us/core; PE busy 83%.
"""

import os
import sys

sys.path.insert(0, "/opt/trn_rl_repo")

import numpy as np

B, S, H, NH, HD = 2, 2048, 1024, 16, 64
NCORES = 8
HPC = 4          # heads per core
COLS = HPC * HD  # 256
KC = S // 128    # 16 k chunks
QB = 1024        # q block width
NQT = S // 128   # 16 global q tiles
SC = 512         # seq chunk for projections

_CACHE = {}


def _build():
    import concourse.mybir as mybir
    import concourse.tile as tile
    from concourse import bacc
    from concourse.masks import make_identity

    f32 = mybir.dt.float32
    f16 = mybir.dt.float16
    Exp = mybir.ActivationFunctionType.Exp

    nc = bacc.Bacc("TRN2", target_bir_lowering=False, debug=False,
                   num_devices=NCORES)

    xT_d = nc.dram_tensor("xT", [H, S], f16, kind="ExternalInput").ap()
    wq_d = nc.dram_tensor("wq", [H, COLS], f16, kind="ExternalInput").ap()
    wk_d = nc.dram_tensor("wk", [H, COLS], f16, kind="ExternalInput").ap()
    wv_d = nc.dram_tensor("wv", [H, COLS], f16, kind="ExternalInput").ap()
    wo_d = nc.dram_tensor("wo", [COLS, H], f16, kind="ExternalInput").ap()
    bq_d = nc.dram_tensor("bq", [COLS], f32, kind="ExternalInput").ap()
    bk_d = nc.dram_tensor("bk", [COLS], f32, kind="ExternalInput").ap()
    mask_d = nc.dram_tensor("mask", [S], f32, kind="ExternalInput").ap()
    out_d = nc.dram_tensor("out", [S, H], f32, kind="ExternalOutput").ap()

    with tile.TileContext(nc) as tc:
        pers = tc.alloc_tile_pool(name="pers", bufs=1)
        psA = tc.alloc_tile_pool(name="psA", bufs=2, space="PSUM")
        psB = tc.alloc_tile_pool(name="psB", bufs=2, space="PSUM")
        work = tc.alloc_tile_pool(name="work", bufs=3)

        qT = [pers.tile([128, S], f16, tag=f"qT{i}", name=f"qT{i}")
              for i in range(2)]
        kT = [pers.tile([128, S], f16, tag=f"kT{i}", name=f"kT{i}")
              for i in range(2)]
        vt = [pers.tile([128, HPC * 65], f16, tag=f"v{i}", name=f"v{i}")
              for i in range(KC)]
        asm = [pers.tile([128, COLS], f16, tag=f"asm{i}", name=f"asm{i}")
               for i in range(NQT)]
        xt4 = [pers.tile([128, 2 * S], f16, tag=f"xt4{i}", name=f"xt4{i}")
               for i in range(4)]
        wq_a = pers.tile([128, 2048], f16, tag="wq", name="wq_a")
        wk_a = pers.tile([128, 2048], f16, tag="wk", name="wk_a")
        wv_a = pers.tile([128, 2048], f16, tag="wv", name="wv_a")
        wo_a = pers.tile([128, 2048], f16, tag="wo", name="wo_a")

        def xT(hc):
            """View of H-chunk hc of x^T: [128, S] slice of a packed tile."""
            return xt4[hc // 2][:, (hc % 2) * S:(hc % 2) * S + S]
        bq_s = pers.tile([128, 2], f32, tag="bq", name="bq_s")
        bk_s = pers.tile([128, 2], f32, tag="bk", name="bk_s")
        mask_s = pers.tile([128, KC], f32, tag="mask", name="mask_s")
        id65 = pers.tile([65, 65], f32, tag="id65", name="id65")
        id128 = pers.tile([128, 128], f16, tag="id128", name="id128")

        warm = pers.tile([1, 1], f32, tag="warm", name="warm")
        nc.gpsimd.memset(warm[:], 0.0)
        nc.scalar.activation(warm[:], warm[:], Exp)
        make_identity(nc, id65[:])
        make_identity(nc, id128[:])

        # Few large DMAs on one HWDGE queue (dispatch is ~650ns/DMA, so
        # batch aggressively), ordered so the projection pipeline starts as
        # early as possible (queue order = arrival order).
        def xt4_pair(t, lo, hi):
            out = xt4[t].rearrange("p (c s) -> p c s", c=2)[:, :, lo:hi]
            in_ = xT_d[t * 256:(t + 1) * 256, lo:hi].rearrange(
                "(c p) s -> p c s", p=128)
            nc.sync.dma_start(out, in_)

        nc.sync.dma_start(wq_a.rearrange("p (c n) -> p c n", c=8),
                          wq_d.rearrange("(c p) n -> p c n", p=128))
        for t in range(4):
            xt4_pair(t, 0, SC)
        nc.sync.dma_start(wk_a.rearrange("p (c n) -> p c n", c=8),
                          wk_d.rearrange("(c p) n -> p c n", p=128))
        nc.sync.dma_start(wv_a.rearrange("p (c n) -> p c n", c=8),
                          wv_d.rearrange("(c p) n -> p c n", p=128))
        nc.sync.dma_start(bq_s[:], bq_d.rearrange("(a p) -> p a", p=128))
        nc.sync.dma_start(bk_s[:], bk_d.rearrange("(a p) -> p a", p=128))
        nc.sync.dma_start(mask_s[:], mask_d.rearrange("(a p) -> p a", p=128))
        for t in range(4):
            xt4_pair(t, SC, S)
        nc.sync.dma_start(wo_a.rearrange("p (c n) -> p c n", c=2),
                          wo_d.rearrange("(c p) n -> p c n", p=128))

        Ident = mybir.ActivationFunctionType.Identity

        def qk_proj(w_a, b_s, dst, pi, sc, act=False):
            ps = psA.tile([128, SC], f32, tag="sc", name="pps")
            for hc in range(8):
                nc.tensor.matmul(
                    ps[:], w_a[:, hc * COLS + pi * 128:hc * COLS + pi * 128 + 128],
                    xT(hc)[:, sc * SC:(sc + 1) * SC],
                    start=(hc == 0), stop=(hc == 7))
            if act:
                # phase-1: ACT is idle and DVE gates the scores critical path
                nc.scalar.activation(dst[pi][:, sc * SC:(sc + 1) * SC], ps[:],
                                     Ident, bias=b_s[:, pi:pi + 1])
            else:
                nc.vector.tensor_scalar_add(dst[pi][:, sc * SC:(sc + 1) * SC],
                                            ps[:], b_s[:, pi:pi + 1])

        def v_proj(st):
            ps = psB.tile([128, COLS], f32, tag="cx", name="vps")
            for hc in range(8):
                nc.tensor.matmul(ps[:], xT(hc)[:, st * 128:(st + 1) * 128],
                                 wv_a[:, hc * COLS:(hc + 1) * COLS],
                                 start=(hc == 0), stop=(hc == 7))
            nc.vector.memset(vt[st][:], 1.0)
            nc.vector.tensor_copy(
                vt[st].rearrange("p (h c) -> p h c", c=65)[:, :, 0:64],
                ps[:].rearrange("p (h c) -> p h c", c=64))

        # ---- attention machinery ----
        ctx_open = {}   # (hp, qb4) -> open PSUM accumulator
        ctx_done = {}   # (hp, qb4) -> SBUF ctx ready for normalize
        pend = []       # global pending ctx matmuls (software pipeline)

        def emit_ctx(ctx_ps, hp, kc, ex):
            for j in range(2):
                h = hp * 2 + j
                nc.tensor.matmul(ctx_ps[:, j * 512:(j + 1) * 512],
                                 vt[kc][:, h * 65:(h + 1) * 65],
                                 ex[:, j * 512:(j + 1) * 512],
                                 start=(kc == 0), stop=(kc == KC - 1))

        def attn(hp, qb4, kcs):
            """Emit scores+exp for the given kcs of combo (hp, qb4); ctx
            matmuls are deferred through a global 2-deep pipeline so the
            in-order PE never waits on the exp they consume. scores/exp
            tiles are [128, 1024] = [h0 512q | h1 512q]."""
            key = (hp, qb4)
            if key not in ctx_open:
                ctx_open[key] = psB.tile([65, QB], f32, tag="cx",
                                         name=f"ctx{hp}_{qb4}")
            ctx_ps = ctx_open[key]
            qs = qb4 * 512
            for kc in kcs:
                sc_ps = psA.tile([128, QB], f32, tag="sc", name="sc_ps")
                for j in range(2):
                    nc.tensor.matmul(
                        sc_ps[:, j * 512:(j + 1) * 512],
                        kT[hp][j * 64:j * 64 + 64, kc * 128:(kc + 1) * 128],
                        qT[hp][j * 64:j * 64 + 64, qs:qs + 512],
                        start=True, stop=True)
                ex = work.tile([128, QB], f16, tag="exp", name="exp", bufs=10)
                nc.scalar.activation(ex[:], sc_ps[:], Exp,
                                     bias=mask_s[:, kc:kc + 1], scale=0.125)
                while len(pend) >= 5:
                    emit_ctx(*pend.pop(0))
                pend.append((ctx_ps, hp, kc, ex))

        def finish_copy(hp, qb4):
            """Drain the finished ctx accumulator to SBUF, freeing its PSUM
            slot for the next pair."""
            ctx_ps = ctx_open.pop((hp, qb4))
            for it in [p for p in pend if p[0] is ctx_ps]:
                pend.remove(it)
                emit_ctx(*it)
            ctx_sb = work.tile([65, QB], f32, tag="ctxsb", name="ctx_sb", bufs=4)
            nc.vector.tensor_copy(ctx_sb[:], ctx_ps[:])
            ctx_done[(hp, qb4)] = ctx_sb

        def finish_norm(hp, qb4):
            ctx_sb = ctx_done.pop((hp, qb4))
            for j in range(2):
                h = hp * 2 + j
                # pack the 4 q-tile transposes into one PSUM bank, one strided
                # reciprocal, then 4 per-partition normalize-muls.
                t1p = psB.tile([128, 260], f32, tag="cx", name="t1p")
                for qt in range(4):
                    nc.tensor.transpose(
                        t1p[:, qt * 65:(qt + 1) * 65],
                        ctx_sb[:, j * 512 + qt * 128:j * 512 + (qt + 1) * 128],
                        id65[:])
                rc4 = work.tile([128, 4], f32, tag="rc", name="rc")
                nc.vector.reciprocal(
                    rc4[:], t1p.rearrange("p (q c) -> p q c", c=65)[:, :, 64])
                for qt in range(4):
                    nc.vector.tensor_scalar_mul(
                        asm[qb4 * 4 + qt][:, h * 64:(h + 1) * 64],
                        t1p[:, qt * 65:qt * 65 + 64], rc4[:, qt:qt + 1])

        def tail(qb4, qts=range(4), act=False):
            # act=True (final tail, ACT idle): ctn on ACT, ob on DVE so the
            # per-qt chains pipeline across three engines.
            cp_ctn = nc.scalar.copy if act else nc.vector.tensor_copy
            cp_ob = nc.vector.tensor_copy
            for qt in qts:
                gqt = qb4 * 4 + qt
                t2p = psB.tile([128, 256], f16, tag="cx", name="t2p")
                for cc in range(2):
                    nc.tensor.transpose(
                        t2p[:, cc * 128:(cc + 1) * 128],
                        asm[gqt][:, cc * 128:(cc + 1) * 128], id128[:])
                ctn = work.tile([128, 256], f16, tag="ctn", name="ctn", bufs=4)
                cp_ctn(ctn[:], t2p[:])
                op = psB.tile([128, H], f32, tag="cx", name="op")
                for cc in range(2):
                    for fj in range(2):
                        nc.tensor.matmul(op[:, fj * 512:(fj + 1) * 512],
                                         ctn[:, cc * 128:(cc + 1) * 128],
                                         wo_a[:, cc * H + fj * 512:cc * H + (fj + 1) * 512],
                                         start=(cc == 0), stop=(cc == 1))
                ob = work.tile([128, H], f32, tag="ob", name="ob", bufs=4)
                cp_ob(ob[:], op[:])
                nc.sync.dma_start(out_d[gqt * 128:(gqt + 1) * 128, :], ob[:])

        def proj_q(sc):
            for pi in range(2):
                qk_proj(wq_a, bq_s, qT, pi, sc)

        # ---- schedule ----
        # Phase 1: kv projections at single-kc interleave with attention on
        # the first q-chunk for both head pairs. q projections for later
        # q-chunks are deferred into phase 2 as PE filler while ACT is the
        # bottleneck.
        for pi in range(2):
            qk_proj(wq_a, bq_s, qT, pi, 0, act=True)
        for sc in range(4):
            for pi in range(2):
                qk_proj(wk_a, bk_s, kT, pi, sc, act=True)
            for i in range(4):
                kc = sc * 4 + i
                v_proj(kc)
                attn(0, 0, range(kc, kc + 1))
        proj_q(1)

        # Phase 2: one head-pair in flight at a time; the second cx PSUM slot
        # rotates between the previous pairs' normalize-transposes and the
        # output projection, interleaved into each pair's kc loop in small
        # items so the PE always has ready work while ACT chews through exp.
        def emit_item(it):
            kind, arg = it
            if kind == "n":
                finish_norm(*arg)
            elif kind == "t":
                tail(arg[0], qts=[arg[1]])
            else:
                qk_proj(wq_a, bq_s, qT, arg[0], arg[1])

        plan = [
            ((1, 0), [("n", (0, 0)), ("q", (0, 2)), ("q", (1, 2))]),
            ((0, 1), [("n", (1, 0)), ("t", (0, 0)), ("t", (0, 1))]),
            ((1, 1), [("t", (0, 2)), ("t", (0, 3)), ("n", (0, 1))]),
            ((0, 2), [("n", (1, 1)), ("q", (0, 3)), ("q", (1, 3))]),
            ((1, 2), [("n", (0, 2)), ("t", (1, 0)), ("t", (1, 1))]),
            ((0, 3), [("t", (1, 2)), ("t", (1, 3)), ("n", (1, 2))]),
            ((1, 3), [("n", (0, 3)), ("t", (2, 0)), ("t", (2, 1)),
                      ("t", (2, 2)), ("t", (2, 3))]),
        ]
        bounds = [(0, 2), (2, 4), (4, 8), (8, 12), (12, 14), (14, KC)]
        prev = (0, 0)
        for (hp, qb4), items in plan:
            cur = 0
            for i, (k0, k1) in enumerate(bounds):
                attn(hp, qb4, range(k0, k1))
                if i == 1 and prev is not None:
                    finish_copy(*prev)
                if i >= 2 and cur < len(items):
                    emit_item(items[cur])
                    cur += 1
            while cur < len(items):
                emit_item(items[cur])
                cur += 1
            prev = (hp, qb4)
        finish_copy(1, 3)
        finish_norm(1, 3)
        tail(3, act=True)

        work.release()
        psB.release()
        psA.release()
        pers.release()

    nc.compile()
    return nc


def _get_nc():
    if "nc" not in _CACHE:
        _CACHE["nc"] = _build()
    return _CACHE["nc"]


def kernel(hidden_states, attention_mask, Wq, bq, Wk, bk, Wv, bv, Wo, bo):
    from concourse.bass_utils import run_bass_kernel_spmd

    hidden_states = np.asarray(hidden_states, np.float32)
    attention_mask = np.asarray(attention_mask, np.float32)
    Wq, Wk, Wv, Wo = (np.asarray(a, np.float32) for a in (Wq, Wk, Wv, Wo))
    bq, bk, bv, bo = (np.asarray(a, np.float32) for a in (bq, bk, bv, bo))

    nc = _get_nc()
    in_maps = []
    xTb = [np.ascontiguousarray(hidden_states[b].T).astype(np.float16)
           for b in range(B)]
    maskb = [np.ascontiguousarray(attention_mask[b, 0, 0, :])
             for b in range(B)]
    for c in range(NCORES):
        b, g = c // HPC, c % HPC
        cs = slice(g * COLS, (g + 1) * COLS)
        in_maps.append({
            "xT": xTb[b],
            "wq": np.ascontiguousarray(Wq[:, cs]).astype(np.float16),
            "wk": np.ascontiguousarray(Wk[:, cs]).astype(np.float16),
            "wv": np.ascontiguousarray(Wv[:, cs]).astype(np.float16),
            "wo": np.ascontiguousarray(Wo[cs, :]).astype(np.float16),
            "bq": np.ascontiguousarray(bq[cs]),
            "bk": np.ascontiguousarray(bk[cs]),
            "mask": maskb[b],
        })

    trace = bool(os.environ.get("KERNEL_TRACE"))
    kw = {}
    if trace:
        kw = dict(trace=True, tmpdir=os.environ.get("KERNEL_TRACE_DIR"))
    res = run_bass_kernel_spmd(nc, in_maps, list(range(NCORES)), **kw)
    _CACHE["last_result"] = res

    out = np.zeros((B, S, H), np.float32)
    for c in range(NCORES):
        out[c // HPC] += res.results[c]["out"]
    out += bv @ Wo + bo
    return out



# revision 2
# speedup vs baseline: 1.0348x; 1.0348x over previous
"""Multi-head attention (B=2, S=2048, H=1024, NH=16 heads of 64) on 8 trn2
NeuronCores, tensor-parallel over heads with batch parallelism on top.

Sharding: core c handles batch b=c//4 and head-group g=c%4 (4 heads, 256 of
the 1024 hidden cols). Each core computes the partial output
ctx_g @ Wo[g_rows, :]; the host sums the 4 partials per batch and adds the
closed-form bias terms (bv @ Wo + bo; bq/bk are applied on-device).

Device math (per core), fp16 matmul operands, fp32 PSUM accumulation:

  qT/kT = Wq_g^T x_b^T (+bias/partition)  [2 head-pair tiles of 128 x 2048]
  v     = x_b Wv_g                        [16 tiles of 128 x (4*65)]; col 64
                                          of each head block = 1.0, so the
                                          ctx matmul also accumulates the
                                          softmax denominators
  scoresT[k,q] = kT.T qT                  (PE; head pairs packed as
                                           [h0 512q | h1 512q] per 2-bank
                                           PSUM tile)
  expT = exp(0.125*scoresT + mask[k])     (one ACT op: scale+mask+exp; mask
                                           is a per-partition bias; no max
                                           subtraction needed -- |scores|<~4)
  ctx[q,c] += expT.T v_aug                (PE, stationary=expT slice
                                           [128k x 128q], moving=v [128k x 65]
                                           -> 65-cycle accumulation steps into
                                           a [128q, 65] PSUM tile per (head,
                                           qtile); K-depth is free on the PE,
                                           so this halves ctx cost vs the
                                           moving-exp form AND lands ctx in
                                           [q, d] layout)
  normalize: strided reciprocal of column 64, per-partition mul -> asm[q,:]
             (no transposes needed)
  out[q,:] = ctx_n @ Wo_g                 (PE, via transpose to [c,q])

Schedule: single-kc software pipelining; phase 1 interleaves the projections
with attention on one head-pair; phase 2 runs one (pair, q-chunk) combo at a
time with the deferred q-projections and output tails spliced in as PE
filler while ACT (the exp engine) is the steady-state bottleneck.
"""

import os
import sys

sys.path.insert(0, "/opt/trn_rl_repo")

import numpy as np

B, S, H, NH, HD = 2, 2048, 1024, 16, 64
NCORES = 8
HPC = 4          # heads per core
COLS = HPC * HD  # 256
KC = S // 128    # 16 k chunks
QB = 1024        # q block width
NQT = S // 128   # 16 global q tiles
SC = 512         # seq chunk for projections
PEND = 5         # ctx software-pipeline depth (kc slots)

_CACHE = {}


def _build():
    import concourse.mybir as mybir
    import concourse.tile as tile
    from concourse import bacc
    from concourse.masks import make_identity

    f32 = mybir.dt.float32
    f16 = mybir.dt.float16
    Exp = mybir.ActivationFunctionType.Exp

    nc = bacc.Bacc("TRN2", target_bir_lowering=False, debug=False,
                   num_devices=NCORES)

    xT_d = nc.dram_tensor("xT", [H, S], f16, kind="ExternalInput").ap()
    wq_d = nc.dram_tensor("wq", [H, COLS], f16, kind="ExternalInput").ap()
    wk_d = nc.dram_tensor("wk", [H, COLS], f16, kind="ExternalInput").ap()
    wv_d = nc.dram_tensor("wv", [H, COLS], f16, kind="ExternalInput").ap()
    wo_d = nc.dram_tensor("wo", [COLS, H], f16, kind="ExternalInput").ap()
    bq_d = nc.dram_tensor("bq", [COLS], f32, kind="ExternalInput").ap()
    bk_d = nc.dram_tensor("bk", [COLS], f32, kind="ExternalInput").ap()
    mask_d = nc.dram_tensor("mask", [S], f32, kind="ExternalInput").ap()
    out_d = nc.dram_tensor("out", [S, H], f32, kind="ExternalOutput").ap()

    with tile.TileContext(nc) as tc:
        pers = tc.alloc_tile_pool(name="pers", bufs=1)
        psA = tc.alloc_tile_pool(name="psA", bufs=2, space="PSUM")
        psC = tc.alloc_tile_pool(name="psC", bufs=2, space="PSUM")
        psT = tc.alloc_tile_pool(name="psT", bufs=2, space="PSUM")
        work = tc.alloc_tile_pool(name="work", bufs=3)

        qT = [pers.tile([128, S], f16, tag=f"qT{i}", name=f"qT{i}")
              for i in range(2)]
        kT = [pers.tile([128, S], f16, tag=f"kT{i}", name=f"kT{i}")
              for i in range(2)]
        vt = [pers.tile([128, HPC * 65], f16, tag=f"v{i}", name=f"v{i}")
              for i in range(KC)]
        asm = [pers.tile([128, COLS], f16, tag=f"asm{i}", name=f"asm{i}")
               for i in range(NQT)]
        xt4 = [pers.tile([128, 2 * S], f16, tag=f"xt4{i}", name=f"xt4{i}")
               for i in range(4)]
        wq_a = pers.tile([128, 2048], f16, tag="wq", name="wq_a")
        wk_a = pers.tile([128, 2048], f16, tag="wk", name="wk_a")
        wv_a = pers.tile([128, 2048], f16, tag="wv", name="wv_a")
        wo_a = pers.tile([128, 2048], f16, tag="wo", name="wo_a")

        def xT(hc):
            """View of H-chunk hc of x^T: [128, S] slice of a packed tile."""
            return xt4[hc // 2][:, (hc % 2) * S:(hc % 2) * S + S]
        bq_s = pers.tile([128, 2], f32, tag="bq", name="bq_s")
        bk_s = pers.tile([128, 2], f32, tag="bk", name="bk_s")
        mask_s = pers.tile([128, KC], f32, tag="mask", name="mask_s")
        id128 = pers.tile([128, 128], f16, tag="id128", name="id128")

        warm = pers.tile([1, 1], f32, tag="warm", name="warm")
        nc.gpsimd.memset(warm[:], 0.0)
        nc.scalar.activation(warm[:], warm[:], Exp)
        make_identity(nc, id128[:])

        # Few large DMAs on one HWDGE queue (dispatch is ~650ns/DMA, so
        # batch aggressively), ordered so the projection pipeline starts as
        # early as possible (queue order = arrival order).
        def xt4_pair(t, lo, hi):
            out = xt4[t].rearrange("p (c s) -> p c s", c=2)[:, :, lo:hi]
            in_ = xT_d[t * 256:(t + 1) * 256, lo:hi].rearrange(
                "(c p) s -> p c s", p=128)
            nc.sync.dma_start(out, in_)

        nc.sync.dma_start(wq_a.rearrange("p (c n) -> p c n", c=8),
                          wq_d.rearrange("(c p) n -> p c n", p=128))
        for t in range(4):
            xt4_pair(t, 0, SC)
        nc.sync.dma_start(wk_a.rearrange("p (c n) -> p c n", c=8),
                          wk_d.rearrange("(c p) n -> p c n", p=128))
        nc.sync.dma_start(wv_a.rearrange("p (c n) -> p c n", c=8),
                          wv_d.rearrange("(c p) n -> p c n", p=128))
        nc.sync.dma_start(bq_s[:], bq_d.rearrange("(a p) -> p a", p=128))
        nc.sync.dma_start(bk_s[:], bk_d.rearrange("(a p) -> p a", p=128))
        nc.sync.dma_start(mask_s[:], mask_d.rearrange("(a p) -> p a", p=128))
        for t in range(4):
            xt4_pair(t, SC, S)
        nc.sync.dma_start(wo_a.rearrange("p (c n) -> p c n", c=2),
                          wo_d.rearrange("(c p) n -> p c n", p=128))

        Ident = mybir.ActivationFunctionType.Identity

        def qk_proj(w_a, b_s, dst, pi, sc, act=False):
            ps = psA.tile([128, SC], f32, tag="sc", name="pps")
            for hc in range(8):
                nc.tensor.matmul(
                    ps[:], w_a[:, hc * COLS + pi * 128:hc * COLS + pi * 128 + 128],
                    xT(hc)[:, sc * SC:(sc + 1) * SC],
                    start=(hc == 0), stop=(hc == 7))
            if act:
                # phase-1: ACT is idle and DVE gates the scores critical path
                nc.scalar.activation(dst[pi][:, sc * SC:(sc + 1) * SC], ps[:],
                                     Ident, bias=b_s[:, pi:pi + 1])
            else:
                nc.vector.tensor_scalar_add(dst[pi][:, sc * SC:(sc + 1) * SC],
                                            ps[:], b_s[:, pi:pi + 1])

        def v_proj(st):
            ps = psT.tile([128, COLS], f32, tag="cx", name="vps")
            for hc in range(8):
                nc.tensor.matmul(ps[:], xT(hc)[:, st * 128:(st + 1) * 128],
                                 wv_a[:, hc * COLS:(hc + 1) * COLS],
                                 start=(hc == 0), stop=(hc == 7))
            nc.vector.memset(vt[st][:], 1.0)
            nc.vector.tensor_copy(
                vt[st].rearrange("p (h c) -> p h c", c=65)[:, :, 0:64],
                ps[:].rearrange("p (h c) -> p h c", c=64))

        # ---- attention machinery ----
        ctx_open = {}   # (hp, qb4) -> [j0, j1] open PSUM accumulators
        pend = []       # global pending ctx matmuls (software pipeline)

        def emit_ctx(hp, qb4, kc, ex):
            key = (hp, qb4)
            if key not in ctx_open:
                # lazy: allocated at first flush so the slot-reuse wait lands
                # after the previous combo's normalize has been emitted
                ctx_open[key] = [
                    psC.tile([128, 260], f32, tag="ctx",
                             name=f"ctx{hp}_{qb4}_{j}") for j in range(2)]
            ctxp = ctx_open[key]
            for j in range(2):
                h = hp * 2 + j
                for qt in range(4):
                    nc.tensor.matmul(
                        ctxp[j][:, qt * 65:(qt + 1) * 65],
                        ex[:, j * 512 + qt * 128:j * 512 + (qt + 1) * 128],
                        vt[kc][:, h * 65:(h + 1) * 65],
                        start=(kc == 0 and qt == 0),
                        stop=(kc == KC - 1 and qt == 3))

        def attn(hp, qb4, kcs):
            """Emit scores+exp for the given kcs of combo (hp, qb4); ctx
            matmuls are deferred through a global pipeline so the in-order PE
            never waits on the exp they consume. scores/exp tiles are
            [128, 1024] = [h0 512q | h1 512q]."""
            qs = qb4 * 512
            for kc in kcs:
                sc_ps = psA.tile([128, QB], f32, tag="sc", name="sc_ps")
                for j in range(2):
                    nc.tensor.matmul(
                        sc_ps[:, j * 512:(j + 1) * 512],
                        kT[hp][j * 64:j * 64 + 64, kc * 128:(kc + 1) * 128],
                        qT[hp][j * 64:j * 64 + 64, qs:qs + 512],
                        start=True, stop=True)
                ex = work.tile([128, QB], f16, tag="exp", name="exp", bufs=8)
                nc.scalar.activation(ex[:], sc_ps[:], Exp,
                                     bias=mask_s[:, kc:kc + 1], scale=0.125)
                while len(pend) >= PEND:
                    emit_ctx(*pend.pop(0))
                pend.append((hp, qb4, kc, ex))

        def finish_norm(hp, qb4):
            """Flush this combo's remaining ctx matmuls, then normalize the
            [128q, 65]-tiled PSUM accumulators straight into asm (per-
            partition reciprocal-mul; no transposes)."""
            for it in [p for p in pend if (p[0], p[1]) == (hp, qb4)]:
                pend.remove(it)
                emit_ctx(*it)
            ctxp = ctx_open.pop((hp, qb4))
            for j in range(2):
                h = hp * 2 + j
                rc4 = work.tile([128, 4], f32, tag="rc", name="rc")
                nc.vector.reciprocal(
                    rc4[:],
                    ctxp[j].rearrange("p (q c) -> p q c", c=65)[:, :, 64])
                for qt in range(4):
                    nc.vector.tensor_scalar_mul(
                        asm[qb4 * 4 + qt][:, h * 64:(h + 1) * 64],
                        ctxp[j][:, qt * 65:qt * 65 + 64], rc4[:, qt:qt + 1])

        def tail(qb4, qts=range(4), act=False):
            # act=True (final tail, ACT idle): ctn on ACT, ob on DVE so the
            # per-qt chains pipeline across three engines.
            cp_ctn = nc.scalar.copy if act else nc.vector.tensor_copy
            for qt in qts:
                gqt = qb4 * 4 + qt
                t2p = psT.tile([128, 256], f16, tag="cx", name="t2p")
                for cc in range(2):
                    nc.tensor.transpose(
                        t2p[:, cc * 128:(cc + 1) * 128],
                        asm[gqt][:, cc * 128:(cc + 1) * 128], id128[:])
                ctn = work.tile([128, 256], f16, tag="ctn", name="ctn", bufs=4)
                cp_ctn(ctn[:], t2p[:])
                ob = work.tile([128, H], f32, tag="ob", name="ob", bufs=4)
                for fj in range(2):
                    op = psT.tile([128, 512], f32, tag="cx", name="op")
                    for cc in range(2):
                        nc.tensor.matmul(
                            op[:], ctn[:, cc * 128:(cc + 1) * 128],
                            wo_a[:, cc * H + fj * 512:cc * H + (fj + 1) * 512],
                            start=(cc == 0), stop=(cc == 1))
                    nc.vector.tensor_copy(ob[:, fj * 512:(fj + 1) * 512], op[:])
                nc.sync.dma_start(out_d[gqt * 128:(gqt + 1) * 128, :], ob[:])

        def proj_q(sc):
            for pi in range(2):
                qk_proj(wq_a, bq_s, qT, pi, sc)

        # ---- schedule ----
        # Phase 1: kv projections at single-kc interleave with attention on
        # the first q-chunk for both head pairs. q projections for later
        # q-chunks are deferred into phase 2 as PE filler while ACT is the
        # bottleneck.
        for pi in range(2):
            qk_proj(wq_a, bq_s, qT, pi, 0, act=True)
        for sc in range(4):
            for pi in range(2):
                qk_proj(wk_a, bk_s, kT, pi, sc, act=True)
            for i in range(4):
                kc = sc * 4 + i
                v_proj(kc)
                attn(0, 0, range(kc, kc + 1))
        proj_q(1)

        # Phase 2: one head-pair in flight at a time; the previous combo's
        # normalize runs at its end (flushing its ctx pipeline), and output
        # projections plus the deferred q-projections are interleaved into
        # each combo's kc loop in small items as PE filler while ACT chews
        # through exp.
        def emit_item(it):
            kind, arg = it
            if kind == "t":
                tail(arg[0], qts=[arg[1]])
            else:
                qk_proj(wq_a, bq_s, qT, arg[0], arg[1])

        plan = [
            ((1, 0), [("q", (0, 2)), ("q", (1, 2))]),
            ((0, 1), [("t", (0, 0)), ("t", (0, 1))]),
            ((1, 1), [("t", (0, 2)), ("t", (0, 3))]),
            ((0, 2), [("q", (0, 3)), ("q", (1, 3))]),
            ((1, 2), [("t", (1, 0)), ("t", (1, 1))]),
            ((0, 3), [("t", (1, 2)), ("t", (1, 3))]),
            ((1, 3), [("t", (2, 0)), ("t", (2, 1)),
                      ("t", (2, 2)), ("t", (2, 3))]),
        ]
        bounds = [(0, 2), (2, 4), (4, 8), (8, 12), (12, 14), (14, KC)]
        prev = (0, 0)
        for (hp, qb4), items in plan:
            finish_norm(*prev)
            cur = 0
            for i, (k0, k1) in enumerate(bounds):
                attn(hp, qb4, range(k0, k1))
                if i >= 2 and cur < len(items):
                    emit_item(items[cur])
                    cur += 1
            while cur < len(items):
                emit_item(items[cur])
                cur += 1
            prev = (hp, qb4)
        finish_norm(1, 3)
        tail(3, act=True)

        work.release()
        psT.release()
        psC.release()
        psA.release()
        pers.release()

    nc.compile()
    return nc


def _get_nc():
    if "nc" not in _CACHE:
        _CACHE["nc"] = _build()
    return _CACHE["nc"]


def kernel(hidden_states, attention_mask, Wq, bq, Wk, bk, Wv, bv, Wo, bo):
    from concourse.bass_utils import run_bass_kernel_spmd

    hidden_states = np.asarray(hidden_states, np.float32)
    attention_mask = np.asarray(attention_mask, np.float32)
    Wq, Wk, Wv, Wo = (np.asarray(a, np.float32) for a in (Wq, Wk, Wv, Wo))
    bq, bk, bv, bo = (np.asarray(a, np.float32) for a in (bq, bk, bv, bo))

    nc = _get_nc()
    in_maps = []
    xTb = [np.ascontiguousarray(hidden_states[b].T).astype(np.float16)
           for b in range(B)]
    maskb = [np.ascontiguousarray(attention_mask[b, 0, 0, :])
             for b in range(B)]
    for c in range(NCORES):
        b, g = c // HPC, c % HPC
        cs = slice(g * COLS, (g + 1) * COLS)
        in_maps.append({
            "xT": xTb[b],
            "wq": np.ascontiguousarray(Wq[:, cs]).astype(np.float16),
            "wk": np.ascontiguousarray(Wk[:, cs]).astype(np.float16),
            "wv": np.ascontiguousarray(Wv[:, cs]).astype(np.float16),
            "wo": np.ascontiguousarray(Wo[cs, :]).astype(np.float16),
            "bq": np.ascontiguousarray(bq[cs]),
            "bk": np.ascontiguousarray(bk[cs]),
            "mask": maskb[b],
        })

    trace = bool(os.environ.get("KERNEL_TRACE"))
    kw = {}
    if trace:
        kw = dict(trace=True, tmpdir=os.environ.get("KERNEL_TRACE_DIR"))
    res = run_bass_kernel_spmd(nc, in_maps, list(range(NCORES)), **kw)
    _CACHE["last_result"] = res

    out = np.zeros((B, S, H), np.float32)
    for c in range(NCORES):
        out[c // HPC] += res.results[c]["out"]
    out += bv @ Wo + bo
    return out
